# revision 6
# baseline (speedup 1.0000x reference)
"""Multi-head attention (B=2, N=2048, D=1024, H=16) on 8 TRN2 NeuronCores.

Sharding: tensor-parallel over heads. Core c owns heads 2c, 2c+1 (a 128-wide
slice of the concat head dim). Each core:
  - projects Q^T, K^T (transposed layout [dh, rows]) and V (natural [rows, dh])
    for its heads, over all B*N=4096 rows, from host-transposed bf16 x^T inputs
  - attention with transposed scores S^T[k, q] = K Q^T (f32r matmuls), exp on
    ScalarE (scale=1/8 folded in, no max-subtract needed: |scores/8| < ~4),
    softmax denominator via an appended ones-column in V (free on TensorE),
  - partial output projection out^T_c = Wo[:, slice] X_c^T  ->  [1024, 4096]
Host sums the 8 partial outputs and adds bo.
"""

import sys

sys.path.insert(0, "/opt/trn_rl_repo")

from contextlib import ExitStack

import ml_dtypes
import numpy as np

import concourse.bass as bass
import concourse.mybir as mybir
import concourse.tile as tile
from concourse import bacc
from concourse.bass_utils import run_bass_kernel_spmd

B, N, D, H, DH = 2, 2048, 1024, 16, 64
R = B * N  # 4096
NC = 8
HPC = H // NC  # 2 heads per core
DHC = HPC * DH  # 128 head dims per core
QT = 512  # query tile (psum bank / fp32 moving max)
KT = 128  # key tile (psum partitions)
NQT = N // QT  # 4
NKT = N // KT  # 16
NRT = R // QT  # 8 row tiles for projections
KC = D // 128  # 8 contraction chunks

f32 = mybir.dt.float32
f32r = mybir.dt.float32r
bf16 = mybir.dt.bfloat16

_cache = {}


def _fold(ap):
    # [D, X] dram -> [128, KC, X] partition-folded view for one-shot DMA
    return ap.rearrange("(a p) m -> p a m", p=128)


def build():
    if "nc" in _cache:
        return _cache["nc"]
    nc = bacc.Bacc("TRN2", target_bir_lowering=False, debug=False, num_devices=NC)
    xq = nc.dram_tensor("xqT", [D, R], bf16, kind="ExternalInput").ap()
    xk = nc.dram_tensor("xkT", [D, R], bf16, kind="ExternalInput").ap()
    xv = nc.dram_tensor("xvT", [D, R], bf16, kind="ExternalInput").ap()
    wq = nc.dram_tensor("wqT", [D, DHC], bf16, kind="ExternalInput").ap()
    wk = nc.dram_tensor("wkT", [D, DHC], bf16, kind="ExternalInput").ap()
    wv = nc.dram_tensor("wvT", [D, DHC], bf16, kind="ExternalInput").ap()
    wo = nc.dram_tensor("woT", [DHC, D], f32, kind="ExternalInput").ap()
    bq = nc.dram_tensor("bq", [1, DHC], bf16, kind="ExternalInput").ap()
    bk = nc.dram_tensor("bk", [1, DHC], bf16, kind="ExternalInput").ap()
    bv = nc.dram_tensor("bv", [1, DHC], bf16, kind="ExternalInput").ap()
    outT = nc.dram_tensor("outT", [D, R], f32, kind="ExternalOutput").ap()

    with tile.TileContext(nc) as tc, ExitStack() as ctx:
        const = ctx.enter_context(tc.tile_pool(name="const", bufs=1))
        xpool = ctx.enter_context(tc.tile_pool(name="x", bufs=3))
        big = ctx.enter_context(tc.tile_pool(name="big", bufs=1))
        ppool = ctx.enter_context(tc.tile_pool(name="p", bufs=3))
        xtp = ctx.enter_context(tc.tile_pool(name="xt", bufs=2))
        opool = ctx.enter_context(tc.tile_pool(name="o", bufs=3))
        npool = ctx.enter_context(tc.tile_pool(name="norm", bufs=2))
        ps_proj = ctx.enter_context(tc.tile_pool(name="psA", bufs=2, space="PSUM"))
        ps_s = ctx.enter_context(tc.tile_pool(name="psS", bufs=2, space="PSUM"))
        ps_pv = ctx.enter_context(tc.tile_pool(name="psPV", bufs=2, space="PSUM"))

        # ---- constants ----
        wq_sb = const.tile([128, KC, DHC], bf16, tag="wq")
        nc.sync.dma_start(wq_sb[:], _fold(wq))
        wk_sb = const.tile([128, KC, DHC], bf16, tag="wk")
        nc.sync.dma_start(wk_sb[:], _fold(wk))
        wv_sb = const.tile([128, KC, DHC], bf16, tag="wv")
        nc.sync.dma_start(wv_sb[:], _fold(wv))
        wo_st = const.tile([128, D], f32, tag="wost")
        nc.sync.dma_start(wo_st[:], wo)
        wo_sb = const.tile([128, D], f32r, tag="wo")
        nc.vector.tensor_copy(wo_sb[:], wo_st[:])
        bq_sb = const.tile([1, DHC], bf16, tag="bq")
        nc.sync.dma_start(bq_sb[:], bq)
        bk_sb = const.tile([1, DHC], bf16, tag="bk")
        nc.sync.dma_start(bk_sb[:], bk)
        bv_sb = const.tile([1, DHC], bf16, tag="bv")
        nc.sync.dma_start(bv_sb[:], bv)
        ones_q = const.tile([1, QT], bf16, tag="onesq")
        nc.vector.memset(ones_q[:], 1.0)
        ones_r = const.tile([1, 128], bf16, tag="onesr")
        nc.vector.memset(ones_r[:], 1.0)

        # ---- persistent activations ----
        qT_sb = big.tile([128, R], f32r, tag="qT")
        kT_sb = big.tile([128, R], f32r, tag="kT")
        # V_aug blocks: per (b, h, kt) a [128 keys, 65] block; col 64 = 1.0
        v_sb = big.tile([128, B * HPC * NKT, 65], f32r, tag="v")
        # ones column (no f32r Memset in ISA: x*0 + 1 via tensor_scalar)
        nc.vector.tensor_scalar(
            v_sb[:, :, 64:65], v_sb[:, :, 64:65], 0.0, 1.0,
            mybir.AluOpType.mult, mybir.AluOpType.add,
        )

        # ---- Q^T / K^T projections: psum[dh2, r] = sum_d W^T[d,dh2] x^T[d,r] ----
        for dst, xdram, w_sb, b_sb in (
            (qT_sb, xq, wq_sb, bq_sb),
            (kT_sb, xk, wk_sb, bk_sb),
        ):
            for rt in range(NRT):
                xt = xpool.tile([128, KC, QT], bf16, tag="xqk")
                nc.sync.dma_start(xt[:], _fold(xdram[:, rt * QT : (rt + 1) * QT]))
                ps = ps_proj.tile([128, QT], f32, tag="proj")
                for kc in range(KC):
                    nc.tensor.matmul(
                        ps[:], w_sb[:, kc, :], xt[:, kc, :],
                        start=(kc == 0), stop=False,
                    )
                nc.tensor.matmul(ps[:], b_sb[:], ones_q[:], start=False, stop=True)
                nc.vector.tensor_copy(dst[:, rt * QT : (rt + 1) * QT], ps[:])

        # ---- V projection (natural layout): psum[r, dh2] = sum_d x^T[d,r] W^T[d,dh2]
        for rt in range(NRT):
            xt = xpool.tile([128, KC, QT], bf16, tag="xv")
            nc.sync.dma_start(xt[:], _fold(xv[:, rt * QT : (rt + 1) * QT]))
            for rs in range(QT // 128):
                ps = ps_proj.tile([128, DHC], f32, tag="proj")
                for kc in range(KC):
                    nc.tensor.matmul(
                        ps[:],
                        xt[:, kc, rs * 128 : (rs + 1) * 128],
                        wv_sb[:, kc, :],
                        start=(kc == 0), stop=False,
                    )
                nc.tensor.matmul(ps[:], ones_r[:], bv_sb[:], start=False, stop=True)
                gt = rt * (QT // 128) + rs  # global 128-row tile index 0..31
                b, kt = gt // NKT, gt % NKT
                for h in range(HPC):
                    blk = (b * HPC + h) * NKT + kt
                    nc.vector.tensor_copy(
                        v_sb[:, blk, 0:64], ps[:, 64 * h : 64 * h + 64]
                    )

        # ---- attention + output projection, per batch ----
        for b in range(B):
            xT = xtp.tile([128, N], f32r, tag="xT")  # X^T for this batch (2 heads)
            for qt in range(NQT):
                qs = slice(b * N + qt * QT, b * N + (qt + 1) * QT)
                pvs = [
                    ps_pv.tile([65, QT], f32, tag="pv", name=f"pv{h}")
                    for h in range(HPC)
                ]
                for kt in range(NKT):
                    ks = slice(b * N + kt * KT, b * N + (kt + 1) * KT)
                    sg = ps_s.tile([128, 2 * QT], f32, tag="sg")
                    for h in range(HPC):
                        hp = slice(64 * h, 64 * h + 64)
                        nc.tensor.matmul(
                            sg[:, h * QT : (h + 1) * QT],
                            kT_sb[hp, ks],
                            qT_sb[hp, qs],
                            start=True, stop=True,
                        )
                    pt = ppool.tile([128, 2 * QT], f32r, tag="p")
                    nc.scalar.activation(
                        pt[:], sg[:], mybir.ActivationFunctionType.Exp, scale=0.125
                    )
                    for h in range(HPC):
                        blk = (b * HPC + h) * NKT + kt
                        nc.tensor.matmul(
                            pvs[h][:],
                            v_sb[:, blk, :],
                            pt[:, h * QT : (h + 1) * QT],
                            start=(kt == 0), stop=(kt == NKT - 1),
                        )
                for h in range(HPC):
                    # sumexp sits at psum partition 64; shift to 0 via sbuf DMA
                    se = npool.tile([65, QT], f32, tag="se")
                    nc.vector.tensor_copy(se[64:65, :], pvs[h][64:65, :])
                    rc = npool.tile([1, QT], f32, tag="rc")
                    nc.sync.dma_start(rc[:], se[64:65, :])
                    nc.vector.reciprocal(rc[:], rc[:])
                    rb = npool.tile([64, QT], f32, tag="rb")
                    nc.gpsimd.partition_broadcast(rb[:], rc[:])
                    if h == 0:
                        nc.vector.tensor_mul(
                            xT[0:64, qt * QT : (qt + 1) * QT], pvs[h][0:64, :], rb[:]
                        )
                    else:
                        tmp = npool.tile([64, QT], f32r, tag="tmp")
                        nc.vector.tensor_mul(tmp[:], pvs[h][0:64, :], rb[:])
                        nc.sync.dma_start(
                            xT[64:128, qt * QT : (qt + 1) * QT], tmp[:]
                        )
            for ot in range(KC):
                for qt in range(NQT):
                    ps = ps_proj.tile([128, QT], f32, tag="proj")
                    nc.tensor.matmul(
                        ps[:],
                        wo_sb[:, ot * 128 : (ot + 1) * 128],
                        xT[:, qt * QT : (qt + 1) * QT],
                        start=True, stop=True,
                    )
                    ob = opool.tile([128, QT], f32, tag="o")
                    nc.vector.tensor_copy(ob[:], ps[:])
                    nc.sync.dma_start(
                        outT[ot * 128 : (ot + 1) * 128, b * N + qt * QT : b * N + (qt + 1) * QT],
                        ob[:],
                    )

    nc.compile()
    _cache["nc"] = nc
    return nc


def kernel(x_q, x_k, x_v, Wq, bq, Wk, bk, Wv, bv, Wo, bo, _trace=False):
    x_q = np.asarray(x_q, dtype=np.float32)
    x_k = np.asarray(x_k, dtype=np.float32)
    x_v = np.asarray(x_v, dtype=np.float32)
    Wq, Wk, Wv, Wo = (np.asarray(w, dtype=np.float32) for w in (Wq, Wk, Wv, Wo))
    bq, bk, bv, bo = (np.asarray(v, dtype=np.float32) for v in (bq, bk, bv, bo))

    bf = ml_dtypes.bfloat16
    xqT = np.ascontiguousarray(x_q.reshape(R, D).T).astype(bf)
    xkT = np.ascontiguousarray(x_k.reshape(R, D).T).astype(bf)
    xvT = np.ascontiguousarray(x_v.reshape(R, D).T).astype(bf)

    in_maps = []
    for c in range(NC):
        s = slice(DHC * c, DHC * (c + 1))
        in_maps.append(
            {
                "xqT": xqT,
                "xkT": xkT,
                "xvT": xvT,
                "wqT": np.ascontiguousarray(Wq[s, :].T).astype(bf),
                "wkT": np.ascontiguousarray(Wk[s, :].T).astype(bf),
                "wvT": np.ascontiguousarray(Wv[s, :].T).astype(bf),
                "woT": np.ascontiguousarray(Wo[:, s].T),
                "bq": bq[s][None, :].astype(bf),
                "bk": bk[s][None, :].astype(bf),
                "bv": bv[s][None, :].astype(bf),
            }
        )

    nc = build()
    res = run_bass_kernel_spmd(nc, in_maps, core_ids=list(range(NC)), trace=_trace)
    total = np.zeros((D, R), dtype=np.float32)
    for c in range(NC):
        total += res.results[c]["outT"]
    out = total.T + bo[None, :]
    if _trace:
        kernel.last_exec_time_ns = res.exec_time_ns
    return out.reshape(B, N, D).astype(np.float32)


# revision 7
# speedup vs baseline: 1.0685x; 1.0685x over previous
"""Multi-head attention (B=2, N=2048, D=1024, H=16) on 8 TRN2 NeuronCores.

Sharding: tensor-parallel over heads. Core c owns heads 2c, 2c+1 (a 128-wide
slice of the concat head dim). Each core:
  - projects Q^T, K^T (transposed layout [dh, rows]) and V (natural [rows, dh])
    for its heads, over all B*N=4096 rows, from host-transposed bf16 x^T inputs
  - attention with transposed scores S^T[k, q] = K Q^T (f32r matmuls), exp on
    ScalarE (scale=1/8 folded in, no max-subtract needed: |scores/8| < ~4),
    softmax denominator via an appended ones-column in V (free on TensorE),
  - partial output projection out^T_c = Wo[:, slice] X_c^T  ->  [1024, 4096]
Host sums the 8 partial outputs and adds bo.

Loop order: batch-0 projections, batch-0 attention, batch-1 projections,
batch-1 attention — so attention starts while the other batch projects.
PV psum accumulators are copied to SBUF immediately (frees the PSUM slot, PE
never idles long enough for the HAM clock-gate to re-throttle); the softmax
normalize chain (reciprocal/broadcast/multiply) runs off the critical path.
"""

import sys

sys.path.insert(0, "/opt/trn_rl_repo")

from contextlib import ExitStack

import ml_dtypes
import numpy as np

import concourse.bass as bass
import concourse.mybir as mybir
import concourse.tile as tile
from concourse import bacc
from concourse.bass_utils import run_bass_kernel_spmd

B, N, D, H, DH = 2, 2048, 1024, 16, 64
R = B * N  # 4096
NC = 8
HPC = H // NC  # 2 heads per core
DHC = HPC * DH  # 128 head dims per core
QT = 512  # query tile (psum bank / fp32 moving max)
KT = 128  # key tile (psum partitions)
NQT = N // QT  # 4
NKT = N // KT  # 16
NBRT = N // QT  # 4 row tiles per batch for projections
KC = D // 128  # 8 contraction chunks

f32 = mybir.dt.float32
f32r = mybir.dt.float32r
bf16 = mybir.dt.bfloat16

_cache = {}


def _fold(ap):
    # [D, X] dram -> [128, KC, X] partition-folded view for one-shot DMA
    return ap.rearrange("(a p) m -> p a m", p=128)


def build():
    if "nc" in _cache:
        return _cache["nc"]
    nc = bacc.Bacc("TRN2", target_bir_lowering=False, debug=False, num_devices=NC)
    xq = nc.dram_tensor("xqT", [D, R], bf16, kind="ExternalInput").ap()
    xk = nc.dram_tensor("xkT", [D, R], bf16, kind="ExternalInput").ap()
    xv = nc.dram_tensor("xvT", [D, R], bf16, kind="ExternalInput").ap()
    wq = nc.dram_tensor("wqT", [D, DHC], bf16, kind="ExternalInput").ap()
    wk = nc.dram_tensor("wkT", [D, DHC], bf16, kind="ExternalInput").ap()
    wv = nc.dram_tensor("wvT", [D, DHC], bf16, kind="ExternalInput").ap()
    wo = nc.dram_tensor("woT", [DHC, D], f32, kind="ExternalInput").ap()
    bq = nc.dram_tensor("bq", [1, DHC], bf16, kind="ExternalInput").ap()
    bk = nc.dram_tensor("bk", [1, DHC], bf16, kind="ExternalInput").ap()
    bv = nc.dram_tensor("bv", [1, DHC], bf16, kind="ExternalInput").ap()
    outT = nc.dram_tensor("outT", [D, R], f32, kind="ExternalOutput").ap()

    with tile.TileContext(nc) as tc, ExitStack() as ctx:
        const = ctx.enter_context(tc.tile_pool(name="const", bufs=1))
        xpool = ctx.enter_context(tc.tile_pool(name="x", bufs=3))
        big = ctx.enter_context(tc.tile_pool(name="big", bufs=1))
        ppool = ctx.enter_context(tc.tile_pool(name="p", bufs=3))
        opool = ctx.enter_context(tc.tile_pool(name="o", bufs=3))
        npool = ctx.enter_context(tc.tile_pool(name="norm", bufs=2))
        ps_proj = ctx.enter_context(tc.tile_pool(name="psA", bufs=2, space="PSUM"))
        ps_s = ctx.enter_context(tc.tile_pool(name="psS", bufs=2, space="PSUM"))
        ps_pv = ctx.enter_context(tc.tile_pool(name="psPV", bufs=2, space="PSUM"))

        # ---- constants ----
        wq_sb = const.tile([128, KC, DHC], bf16, tag="wq")
        nc.sync.dma_start(wq_sb[:], _fold(wq))
        wk_sb = const.tile([128, KC, DHC], bf16, tag="wk")
        nc.sync.dma_start(wk_sb[:], _fold(wk))
        wv_sb = const.tile([128, KC, DHC], bf16, tag="wv")
        nc.sync.dma_start(wv_sb[:], _fold(wv))
        wo_st = const.tile([128, D], f32, tag="wost")
        nc.sync.dma_start(wo_st[:], wo)
        wo_sb = const.tile([128, D], f32r, tag="wo")
        nc.vector.tensor_copy(wo_sb[:], wo_st[:])
        bq_sb = const.tile([1, DHC], bf16, tag="bq")
        nc.sync.dma_start(bq_sb[:], bq)
        bk_sb = const.tile([1, DHC], bf16, tag="bk")
        nc.sync.dma_start(bk_sb[:], bk)
        bv_sb = const.tile([1, DHC], bf16, tag="bv")
        nc.sync.dma_start(bv_sb[:], bv)
        ones_q = const.tile([1, QT], bf16, tag="onesq")
        nc.vector.memset(ones_q[:], 1.0)
        ones_r = const.tile([1, 128], bf16, tag="onesr")
        nc.vector.memset(ones_r[:], 1.0)

        # ---- per-batch persistent activations ----
        qTs, kTs, vs = [], [], []
        for b in range(B):
            qTs.append(big.tile([128, N], f32r, tag=f"qT{b}", name=f"qT{b}"))
            kTs.append(big.tile([128, N], f32r, tag=f"kT{b}", name=f"kT{b}"))
            v = big.tile([128, HPC * NKT, 65], f32r, tag=f"v{b}", name=f"v{b}")
            # ones column (no f32r Memset in ISA: x*0 + 1 via tensor_scalar)
            nc.vector.tensor_scalar(
                v[:, :, 64:65], v[:, :, 64:65], 0.0, 1.0,
                mybir.AluOpType.mult, mybir.AluOpType.add,
            )
            vs.append(v)

        def proj_qk(b):
            # psum[dh2, r] = sum_d W^T[d, dh2] x^T[d, r]  (+ bias via K=1 mm)
            for dst, xdram, w_sb, b_sb in (
                (qTs[b], xq, wq_sb, bq_sb),
                (kTs[b], xk, wk_sb, bk_sb),
            ):
                for rt in range(NBRT):
                    rlo = b * N + rt * QT
                    xt = xpool.tile([128, KC, QT], bf16, tag="xqk", name="xqk")
                    nc.sync.dma_start(xt[:], _fold(xdram[:, rlo : rlo + QT]))
                    ps = ps_proj.tile([128, QT], f32, tag="proj", name="psqk")
                    for kc in range(KC):
                        nc.tensor.matmul(
                            ps[:], w_sb[:, kc, :], xt[:, kc, :],
                            start=(kc == 0), stop=False,
                        )
                    nc.tensor.matmul(
                        ps[:], b_sb[:], ones_q[:], start=False, stop=True
                    )
                    nc.vector.tensor_copy(dst[:, rt * QT : (rt + 1) * QT], ps[:])

        def proj_v(b):
            # natural layout: psum[r, dh2] = sum_d x^T[d, r] W^T[d, dh2]
            for rt in range(NBRT):
                rlo = b * N + rt * QT
                xt = xpool.tile([128, KC, QT], bf16, tag="xv", name="xv")
                nc.sync.dma_start(xt[:], _fold(xv[:, rlo : rlo + QT]))
                for rs in range(QT // 128):
                    ps = ps_proj.tile([128, DHC], f32, tag="proj", name="psv")
                    for kc in range(KC):
                        nc.tensor.matmul(
                            ps[:],
                            xt[:, kc, rs * 128 : (rs + 1) * 128],
                            wv_sb[:, kc, :],
                            start=(kc == 0), stop=False,
                        )
                    nc.tensor.matmul(
                        ps[:], ones_r[:], bv_sb[:], start=False, stop=True
                    )
                    kt = rt * (QT // 128) + rs  # key tile index within batch
                    for h in range(HPC):
                        nc.vector.tensor_copy(
                            vs[b][:, h * NKT + kt, 0:64],
                            ps[:, 64 * h : 64 * h + 64],
                        )

        def attention(b):
            xT = opool.tile([128, N], f32r, tag="xT", name=f"xT{b}", bufs=2)
            for qt in range(NQT):
                qs = slice(qt * QT, (qt + 1) * QT)
                pvs = [
                    ps_pv.tile([65, QT], f32, tag="pv", name=f"pv{h}")
                    for h in range(HPC)
                ]
                for kt in range(NKT):
                    ks = slice(kt * KT, (kt + 1) * KT)
                    sg = ps_s.tile([128, 2 * QT], f32, tag="sg", name="sg")
                    for h in range(HPC):
                        hp = slice(64 * h, 64 * h + 64)
                        nc.tensor.matmul(
                            sg[:, h * QT : (h + 1) * QT],
                            kTs[b][hp, ks],
                            qTs[b][hp, qs],
                            start=True, stop=True,
                        )
                    pt = ppool.tile([128, 2 * QT], f32r, tag="p", name="pt")
                    nc.scalar.activation(
                        pt[:], sg[:], mybir.ActivationFunctionType.Exp, scale=0.125
                    )
                    for h in range(HPC):
                        nc.tensor.matmul(
                            pvs[h][:],
                            vs[b][:, h * NKT + kt, :],
                            pt[:, h * QT : (h + 1) * QT],
                            start=(kt == 0), stop=(kt == NKT - 1),
                        )
                for h in range(HPC):
                    # copy to SBUF immediately -> frees the psum slot so the
                    # next q-tile's PV starts without a long PE stall
                    pvsb = npool.tile([65, QT], f32, tag="pvsb", name=f"pvsb{h}")
                    nc.vector.tensor_copy(pvsb[:], pvs[h][:])
                    # sumexp row sits at partition 64; shift to 0 via sbuf DMA
                    rc = npool.tile([1, QT], f32, tag="rc", name=f"rc{h}")
                    nc.sync.dma_start(rc[:], pvsb[64:65, :])
                    nc.vector.reciprocal(rc[:], rc[:])
                    rb = npool.tile([64, QT], f32, tag="rb", name=f"rb{h}")
                    nc.gpsimd.partition_broadcast(rb[:], rc[:])
                    if h == 0:
                        nc.vector.tensor_mul(xT[0:64, qs], pvsb[0:64, :], rb[:])
                    else:
                        tmp = npool.tile([64, QT], f32r, tag="tmp", name="tmp")
                        nc.vector.tensor_mul(tmp[:], pvsb[0:64, :], rb[:])
                        nc.sync.dma_start(xT[64:128, qs], tmp[:])
                for ot in range(KC):
                    ps = ps_proj.tile([128, QT], f32, tag="proj", name="pso")
                    nc.tensor.matmul(
                        ps[:],
                        wo_sb[:, ot * 128 : (ot + 1) * 128],
                        xT[:, qs],
                        start=True, stop=True,
                    )
                    ob = opool.tile([128, QT], f32, tag="o", name="ob")
                    nc.vector.tensor_copy(ob[:], ps[:])
                    nc.sync.dma_start(
                        outT[
                            ot * 128 : (ot + 1) * 128,
                            b * N + qt * QT : b * N + (qt + 1) * QT,
                        ],
                        ob[:],
                    )

        for b in range(B):
            proj_qk(b)
            proj_v(b)
            attention(b)

    nc.compile()
    _cache["nc"] = nc
    return nc


def kernel(x_q, x_k, x_v, Wq, bq, Wk, bk, Wv, bv, Wo, bo, _trace=False):
    x_q = np.asarray(x_q, dtype=np.float32)
    x_k = np.asarray(x_k, dtype=np.float32)
    x_v = np.asarray(x_v, dtype=np.float32)
    Wq, Wk, Wv, Wo = (np.asarray(w, dtype=np.float32) for w in (Wq, Wk, Wv, Wo))
    bq, bk, bv, bo = (np.asarray(v, dtype=np.float32) for v in (bq, bk, bv, bo))

    bf = ml_dtypes.bfloat16
    xqT = np.ascontiguousarray(x_q.reshape(R, D).T).astype(bf)
    xkT = np.ascontiguousarray(x_k.reshape(R, D).T).astype(bf)
    xvT = np.ascontiguousarray(x_v.reshape(R, D).T).astype(bf)

    in_maps = []
    for c in range(NC):
        s = slice(DHC * c, DHC * (c + 1))
        in_maps.append(
            {
                "xqT": xqT,
                "xkT": xkT,
                "xvT": xvT,
                "wqT": np.ascontiguousarray(Wq[s, :].T).astype(bf),
                "wkT": np.ascontiguousarray(Wk[s, :].T).astype(bf),
                "wvT": np.ascontiguousarray(Wv[s, :].T).astype(bf),
                "woT": np.ascontiguousarray(Wo[:, s].T),
                "bq": bq[s][None, :].astype(bf),
                "bk": bk[s][None, :].astype(bf),
                "bv": bv[s][None, :].astype(bf),
            }
        )

    nc = build()
    res = run_bass_kernel_spmd(nc, in_maps, core_ids=list(range(NC)), trace=_trace)
    total = np.zeros((D, R), dtype=np.float32)
    for c in range(NC):
        total += res.results[c]["outT"]
    out = total.T + bo[None, :]
    if _trace:
        kernel.last_exec_time_ns = res.exec_time_ns
    return out.reshape(B, N, D).astype(np.float32)


# revision 8
# speedup vs baseline: 1.0936x; 1.0235x over previous
"""Multi-head attention (B=2, N=2048, D=1024, H=16) on 8 TRN2 NeuronCores.

Sharding: tensor-parallel over heads. Core c owns heads 2c, 2c+1 (a 128-wide
slice of the concat head dim). Each core:
  - projects Q^T, K^T (transposed layout [dh, rows]) and V (natural [rows, dh])
    for its heads, over all B*N=4096 rows, from host-transposed bf16 x^T inputs
  - attention with transposed scores S^T[k, q] = K Q^T (f32r matmuls), exp on
    ScalarE (scale=1/8 folded in, no max-subtract needed: |scores/8| < ~4),
    softmax denominator via an appended ones-column in V (free on TensorE),
  - partial output projection out^T_c = Wo[:, slice] X_c^T  ->  [1024, 4096]
Host sums the 8 partial outputs and adds bo.

Loop order: batch-0 projections, batch-0 attention, batch-1 projections,
batch-1 attention — so attention starts while the other batch projects.
PV psum accumulators are copied to SBUF immediately (frees the PSUM slot, PE
never idles long enough for the HAM clock-gate to re-throttle); the softmax
normalize chain (reciprocal/broadcast/multiply) runs off the critical path.
"""

import sys

sys.path.insert(0, "/opt/trn_rl_repo")

from contextlib import ExitStack

import ml_dtypes
import numpy as np

import concourse.bass as bass
import concourse.mybir as mybir
import concourse.tile as tile
from concourse import bacc
from concourse.bass_utils import run_bass_kernel_spmd

B, N, D, H, DH = 2, 2048, 1024, 16, 64
R = B * N  # 4096
NC = 8
HPC = H // NC  # 2 heads per core
DHC = HPC * DH  # 128 head dims per core
QT = 512  # query tile (psum bank / fp32 moving max)
KT = 128  # key tile (psum partitions)
NQT = N // QT  # 4
NKT = N // KT  # 16
NBRT = N // QT  # 4 row tiles per batch for projections
KC = D // 128  # 8 contraction chunks

f32 = mybir.dt.float32
f32r = mybir.dt.float32r
bf16 = mybir.dt.bfloat16

_cache = {}


def _fold(ap):
    # [D, X] dram -> [128, KC, X] partition-folded view for one-shot DMA
    return ap.rearrange("(a p) m -> p a m", p=128)


def build():
    if "nc" in _cache:
        return _cache["nc"]
    nc = bacc.Bacc("TRN2", target_bir_lowering=False, debug=False, num_devices=NC)
    xq = nc.dram_tensor("xqT", [D, R], bf16, kind="ExternalInput").ap()
    xk = nc.dram_tensor("xkT", [D, R], bf16, kind="ExternalInput").ap()
    xv = nc.dram_tensor("xvT", [D, R], bf16, kind="ExternalInput").ap()
    wq = nc.dram_tensor("wqT", [D, DHC], bf16, kind="ExternalInput").ap()
    wk = nc.dram_tensor("wkT", [D, DHC], bf16, kind="ExternalInput").ap()
    wv = nc.dram_tensor("wvT", [D, DHC], bf16, kind="ExternalInput").ap()
    wo = nc.dram_tensor("woT", [DHC, D], f32, kind="ExternalInput").ap()
    bq = nc.dram_tensor("bq", [1, DHC], bf16, kind="ExternalInput").ap()
    bk = nc.dram_tensor("bk", [1, DHC], bf16, kind="ExternalInput").ap()
    bv = nc.dram_tensor("bv", [1, DHC], bf16, kind="ExternalInput").ap()
    outT = nc.dram_tensor("outT", [D, R], f32, kind="ExternalOutput").ap()

    with tile.TileContext(nc) as tc, ExitStack() as ctx:
        const = ctx.enter_context(tc.tile_pool(name="const", bufs=1))
        xpool = ctx.enter_context(tc.tile_pool(name="x", bufs=3))
        big = ctx.enter_context(tc.tile_pool(name="big", bufs=1))
        ppool = ctx.enter_context(tc.tile_pool(name="p", bufs=3))
        opool = ctx.enter_context(tc.tile_pool(name="o", bufs=3))
        npool = ctx.enter_context(tc.tile_pool(name="norm", bufs=2))
        ps_proj = ctx.enter_context(tc.tile_pool(name="psA", bufs=2, space="PSUM"))
        ps_s = ctx.enter_context(tc.tile_pool(name="psS", bufs=2, space="PSUM"))
        ps_pv = ctx.enter_context(tc.tile_pool(name="psPV", bufs=2, space="PSUM"))

        # ---- constants ----
        wq_sb = const.tile([128, KC, DHC], bf16, tag="wq")
        nc.sync.dma_start(wq_sb[:], _fold(wq))
        wk_sb = const.tile([128, KC, DHC], bf16, tag="wk")
        nc.sync.dma_start(wk_sb[:], _fold(wk))
        wv_sb = const.tile([128, KC, DHC], bf16, tag="wv")
        nc.sync.dma_start(wv_sb[:], _fold(wv))
        wo_st = const.tile([128, D], f32, tag="wost")
        nc.sync.dma_start(wo_st[:], wo)
        wo_sb = const.tile([128, D], f32r, tag="wo")
        nc.vector.tensor_copy(wo_sb[:], wo_st[:])
        bq_sb = const.tile([1, DHC], bf16, tag="bq")
        nc.sync.dma_start(bq_sb[:], bq)
        bk_sb = const.tile([1, DHC], bf16, tag="bk")
        nc.sync.dma_start(bk_sb[:], bk)
        bv_sb = const.tile([1, DHC], bf16, tag="bv")
        nc.sync.dma_start(bv_sb[:], bv)
        ones_q = const.tile([1, QT], bf16, tag="onesq")
        nc.vector.memset(ones_q[:], 1.0)
        ones_r = const.tile([1, 128], bf16, tag="onesr")
        nc.vector.memset(ones_r[:], 1.0)

        # ---- per-batch persistent activations ----
        qTs, kTs, vs = [], [], []
        for b in range(B):
            qTs.append(big.tile([128, N], f32r, tag=f"qT{b}", name=f"qT{b}"))
            kTs.append(big.tile([128, N], f32r, tag=f"kT{b}", name=f"kT{b}"))
            v = big.tile([128, HPC * NKT, 65], f32r, tag=f"v{b}", name=f"v{b}")
            # ones column (no f32r Memset in ISA: x*0 + 1 via tensor_scalar)
            nc.vector.tensor_scalar(
                v[:, :, 64:65], v[:, :, 64:65], 0.0, 1.0,
                mybir.AluOpType.mult, mybir.AluOpType.add,
            )
            vs.append(v)

        def proj_qk(b):
            # psum[dh2, r] = sum_d W^T[d, dh2] x^T[d, r]  (+ bias via K=1 mm)
            for dst, xdram, w_sb, b_sb in (
                (qTs[b], xq, wq_sb, bq_sb),
                (kTs[b], xk, wk_sb, bk_sb),
            ):
                for rt in range(NBRT):
                    rlo = b * N + rt * QT
                    xt = xpool.tile([128, KC, QT], bf16, tag="xqk", name="xqk")
                    nc.sync.dma_start(xt[:], _fold(xdram[:, rlo : rlo + QT]))
                    ps = ps_proj.tile([128, QT], f32, tag="proj", name="psqk")
                    for kc in range(KC):
                        nc.tensor.matmul(
                            ps[:], w_sb[:, kc, :], xt[:, kc, :],
                            start=(kc == 0), stop=False,
                        )
                    nc.tensor.matmul(
                        ps[:], b_sb[:], ones_q[:], start=False, stop=True
                    )
                    nc.vector.tensor_copy(dst[:, rt * QT : (rt + 1) * QT], ps[:])

        def proj_v(b):
            # natural layout: psum[r, dh2] = sum_d x^T[d, r] W^T[d, dh2]
            for rt in range(NBRT):
                rlo = b * N + rt * QT
                xt = xpool.tile([128, KC, QT], bf16, tag="xv", name="xv")
                nc.sync.dma_start(xt[:], _fold(xv[:, rlo : rlo + QT]))
                for rs in range(QT // 128):
                    ps = ps_proj.tile([128, DHC], f32, tag="proj", name="psv")
                    for kc in range(KC):
                        nc.tensor.matmul(
                            ps[:],
                            xt[:, kc, rs * 128 : (rs + 1) * 128],
                            wv_sb[:, kc, :],
                            start=(kc == 0), stop=False,
                        )
                    nc.tensor.matmul(
                        ps[:], ones_r[:], bv_sb[:], start=False, stop=True
                    )
                    kt = rt * (QT // 128) + rs  # key tile index within batch
                    for h in range(HPC):
                        nc.vector.tensor_copy(
                            vs[b][:, h * NKT + kt, 0:64],
                            ps[:, 64 * h : 64 * h + 64],
                        )

        def attention(b, xT, qts):
            for qt in qts:
                qs = slice(qt * QT, (qt + 1) * QT)
                pvs = [
                    ps_pv.tile([65, QT], f32, tag="pv", name=f"pv{h}")
                    for h in range(HPC)
                ]
                for kt in range(NKT):
                    ks = slice(kt * KT, (kt + 1) * KT)
                    sg = ps_s.tile([128, 2 * QT], f32, tag="sg", name="sg")
                    for h in range(HPC):
                        hp = slice(64 * h, 64 * h + 64)
                        nc.tensor.matmul(
                            sg[:, h * QT : (h + 1) * QT],
                            kTs[b][hp, ks],
                            qTs[b][hp, qs],
                            start=True, stop=True,
                        )
                    pt = ppool.tile([128, 2 * QT], f32r, tag="p", name="pt")
                    nc.scalar.activation(
                        pt[:], sg[:], mybir.ActivationFunctionType.Exp, scale=0.125
                    )
                    for h in range(HPC):
                        nc.tensor.matmul(
                            pvs[h][:],
                            vs[b][:, h * NKT + kt, :],
                            pt[:, h * QT : (h + 1) * QT],
                            start=(kt == 0), stop=(kt == NKT - 1),
                        )
                for h in range(HPC):
                    # copy to SBUF immediately -> frees the psum slot so the
                    # next q-tile's PV starts without a long PE stall
                    pvsb = npool.tile([65, QT], f32, tag="pvsb", name=f"pvsb{h}")
                    nc.vector.tensor_copy(pvsb[:], pvs[h][:])
                    # sumexp row sits at partition 64; shift to 0 via sbuf DMA
                    rc = npool.tile([1, QT], f32, tag="rc", name=f"rc{h}")
                    nc.sync.dma_start(rc[:], pvsb[64:65, :])
                    nc.vector.reciprocal(rc[:], rc[:])
                    rb = npool.tile([64, QT], f32, tag="rb", name=f"rb{h}")
                    nc.gpsimd.partition_broadcast(rb[:], rc[:])
                    if h == 0:
                        nc.vector.tensor_mul(xT[0:64, qs], pvsb[0:64, :], rb[:])
                    else:
                        tmp = npool.tile([64, QT], f32r, tag="tmp", name="tmp")
                        nc.vector.tensor_mul(tmp[:], pvsb[0:64, :], rb[:])
                        nc.sync.dma_start(xT[64:128, qs], tmp[:])
                for ot in range(KC):
                    ps = ps_proj.tile([128, QT], f32, tag="proj", name="pso")
                    nc.tensor.matmul(
                        ps[:],
                        wo_sb[:, ot * 128 : (ot + 1) * 128],
                        xT[:, qs],
                        start=True, stop=True,
                    )
                    ob = opool.tile([128, QT], f32, tag="o", name="ob")
                    nc.vector.tensor_copy(ob[:], ps[:])
                    nc.gpsimd.dma_start(
                        outT[
                            ot * 128 : (ot + 1) * 128,
                            b * N + qt * QT : b * N + (qt + 1) * QT,
                        ],
                        ob[:],
                    )

        xTs = [
            opool.tile([128, N], f32r, tag="xT", name=f"xT{b}", bufs=2)
            for b in range(B)
        ]
        proj_qk(0)
        proj_v(0)
        attention(0, xTs[0], range(0, 2))
        proj_qk(1)
        proj_v(1)
        attention(0, xTs[0], range(2, NQT))
        attention(1, xTs[1], range(0, NQT))

    nc.compile()
    _cache["nc"] = nc
    return nc


def kernel(x_q, x_k, x_v, Wq, bq, Wk, bk, Wv, bv, Wo, bo, _trace=False):
    x_q = np.asarray(x_q, dtype=np.float32)
    x_k = np.asarray(x_k, dtype=np.float32)
    x_v = np.asarray(x_v, dtype=np.float32)
    Wq, Wk, Wv, Wo = (np.asarray(w, dtype=np.float32) for w in (Wq, Wk, Wv, Wo))
    bq, bk, bv, bo = (np.asarray(v, dtype=np.float32) for v in (bq, bk, bv, bo))

    bf = ml_dtypes.bfloat16
    xqT = np.ascontiguousarray(x_q.reshape(R, D).T).astype(bf)
    xkT = np.ascontiguousarray(x_k.reshape(R, D).T).astype(bf)
    xvT = np.ascontiguousarray(x_v.reshape(R, D).T).astype(bf)

    in_maps = []
    for c in range(NC):
        s = slice(DHC * c, DHC * (c + 1))
        in_maps.append(
            {
                "xqT": xqT,
                "xkT": xkT,
                "xvT": xvT,
                "wqT": np.ascontiguousarray(Wq[s, :].T).astype(bf),
                "wkT": np.ascontiguousarray(Wk[s, :].T).astype(bf),
                "wvT": np.ascontiguousarray(Wv[s, :].T).astype(bf),
                "woT": np.ascontiguousarray(Wo[:, s].T),
                "bq": bq[s][None, :].astype(bf),
                "bk": bk[s][None, :].astype(bf),
                "bv": bv[s][None, :].astype(bf),
            }
        )

    nc = build()
    res = run_bass_kernel_spmd(nc, in_maps, core_ids=list(range(NC)), trace=_trace)
    total = np.zeros((D, R), dtype=np.float32)
    for c in range(NC):
        total += res.results[c]["outT"]
    out = total.T + bo[None, :]
    if _trace:
        kernel.last_exec_time_ns = res.exec_time_ns
    return out.reshape(B, N, D).astype(np.float32)


# revision 9
# speedup vs baseline: 1.1852x; 1.0837x over previous
"""Multi-head attention (B=2, N=2048, D=1024, H=16) on 8 TRN2 NeuronCores.

Sharding: tensor-parallel over heads. Core c owns heads 2c, 2c+1 (a 128-wide
slice of the concat head dim). Each core:
  - projects Q^T, K^T (transposed layout [dh, rows]) and V (natural [rows, dh])
    for its heads, over all B*N=4096 rows, from host-transposed bf16 x^T inputs
  - attention with transposed scores S^T[k, q] = K Q^T (f32r matmuls), exp on
    ScalarE (scale=1/8 folded in, no max-subtract needed: |scores/8| < ~4),
    softmax denominator via an appended ones-column in V (free on TensorE),
  - partial output projection out^T_c = Wo[:, slice] X_c^T  ->  [1024, 4096]
Host sums the 8 partial outputs and adds bo.

Loop order: batch-0 projections, batch-0 attention, batch-1 projections,
batch-1 attention — so attention starts while the other batch projects.
PV psum accumulators are copied to SBUF immediately (frees the PSUM slot, PE
never idles long enough for the HAM clock-gate to re-throttle); the softmax
normalize chain (reciprocal/broadcast/multiply) runs off the critical path.
"""

import sys

sys.path.insert(0, "/opt/trn_rl_repo")

from contextlib import ExitStack

import ml_dtypes
import numpy as np

import concourse.bass as bass
import concourse.mybir as mybir
import concourse.tile as tile
from concourse import bacc
from concourse.bass_utils import run_bass_kernel_spmd

B, N, D, H, DH = 2, 2048, 1024, 16, 64
R = B * N  # 4096
NC = 8
HPC = H // NC  # 2 heads per core
DHC = HPC * DH  # 128 head dims per core
QT = 512  # query tile (psum bank / fp32 moving max)
KT = 128  # key tile (psum partitions)
NQT = N // QT  # 4
NKT = N // KT  # 16
NBRT = N // QT  # 4 row tiles per batch for projections
KC = D // 128  # 8 contraction chunks

f32 = mybir.dt.float32
f32r = mybir.dt.float32r
bf16 = mybir.dt.bfloat16

_cache = {}


def _fold(ap):
    # [D, X] dram -> [128, KC, X] partition-folded view for one-shot DMA
    return ap.rearrange("(a p) m -> p a m", p=128)


def build():
    if "nc" in _cache:
        return _cache["nc"]
    nc = bacc.Bacc("TRN2", target_bir_lowering=False, debug=False, num_devices=NC)
    xq = nc.dram_tensor("xqT", [D, R], bf16, kind="ExternalInput").ap()
    xk = nc.dram_tensor("xkT", [D, R], bf16, kind="ExternalInput").ap()
    xv = nc.dram_tensor("xvT", [D, R], bf16, kind="ExternalInput").ap()
    wq = nc.dram_tensor("wqT", [D, DHC], bf16, kind="ExternalInput").ap()
    wk = nc.dram_tensor("wkT", [D, DHC], bf16, kind="ExternalInput").ap()
    wv = nc.dram_tensor("wvT", [D, DHC], bf16, kind="ExternalInput").ap()
    wo = nc.dram_tensor("woT", [DHC, D], bf16, kind="ExternalInput").ap()
    bq = nc.dram_tensor("bq", [1, DHC], bf16, kind="ExternalInput").ap()
    bk = nc.dram_tensor("bk", [1, DHC], bf16, kind="ExternalInput").ap()
    bv = nc.dram_tensor("bv", [1, DHC], bf16, kind="ExternalInput").ap()
    outT = nc.dram_tensor("outT", [D, R], f32, kind="ExternalOutput").ap()

    with tile.TileContext(nc) as tc, ExitStack() as ctx:
        const = ctx.enter_context(tc.tile_pool(name="const", bufs=1))
        xpool = ctx.enter_context(tc.tile_pool(name="x", bufs=3))
        big = ctx.enter_context(tc.tile_pool(name="big", bufs=1))
        ppool = ctx.enter_context(tc.tile_pool(name="p", bufs=3))
        opool = ctx.enter_context(tc.tile_pool(name="o", bufs=3))
        npool = ctx.enter_context(tc.tile_pool(name="norm", bufs=2))
        ps_proj = ctx.enter_context(tc.tile_pool(name="psA", bufs=2, space="PSUM"))
        ps_s = ctx.enter_context(tc.tile_pool(name="psS", bufs=2, space="PSUM"))
        ps_pv = ctx.enter_context(tc.tile_pool(name="psPV", bufs=2, space="PSUM"))

        # ---- constants ----
        wq_sb = const.tile([128, KC, DHC], bf16, tag="wq")
        nc.sync.dma_start(wq_sb[:], _fold(wq))
        wk_sb = const.tile([128, KC, DHC], bf16, tag="wk")
        nc.sync.dma_start(wk_sb[:], _fold(wk))
        wv_sb = const.tile([128, KC, DHC], bf16, tag="wv")
        nc.sync.dma_start(wv_sb[:], _fold(wv))
        wo_sb = const.tile([128, D], bf16, tag="wo")
        nc.sync.dma_start(wo_sb[:], wo)
        bq_sb = const.tile([1, DHC], bf16, tag="bq")
        nc.sync.dma_start(bq_sb[:], bq)
        bk_sb = const.tile([1, DHC], bf16, tag="bk")
        nc.sync.dma_start(bk_sb[:], bk)
        bv_sb = const.tile([1, DHC], bf16, tag="bv")
        nc.sync.dma_start(bv_sb[:], bv)
        ones_q = const.tile([1, QT], bf16, tag="onesq")
        nc.vector.memset(ones_q[:], 1.0)
        ones_r = const.tile([1, 128], bf16, tag="onesr")
        nc.vector.memset(ones_r[:], 1.0)

        # ---- per-batch persistent activations ----
        qTs, kTs, vs = [], [], []
        for b in range(B):
            qTs.append(big.tile([128, N], bf16, tag=f"qT{b}", name=f"qT{b}"))
            kTs.append(big.tile([128, N], bf16, tag=f"kT{b}", name=f"kT{b}"))
            v = big.tile([128, HPC * NKT, 65], bf16, tag=f"v{b}", name=f"v{b}")
            nc.vector.memset(v[:, :, 64:65], 1.0)
            vs.append(v)

        def proj_qk(b):
            # psum[dh2, r] = sum_d W^T[d, dh2] x^T[d, r]  (+ bias via K=1 mm)
            for dst, xdram, w_sb, b_sb in (
                (qTs[b], xq, wq_sb, bq_sb),
                (kTs[b], xk, wk_sb, bk_sb),
            ):
                for rt in range(NBRT):
                    rlo = b * N + rt * QT
                    xt = xpool.tile([128, KC, QT], bf16, tag="xqk", name="xqk")
                    nc.sync.dma_start(xt[:], _fold(xdram[:, rlo : rlo + QT]))
                    ps = ps_proj.tile([128, QT], f32, tag="proj", name="psqk")
                    for kc in range(KC):
                        nc.tensor.matmul(
                            ps[:], w_sb[:, kc, :], xt[:, kc, :],
                            start=(kc == 0), stop=False,
                        )
                    nc.tensor.matmul(
                        ps[:], b_sb[:], ones_q[:], start=False, stop=True
                    )
                    nc.vector.tensor_copy(dst[:, rt * QT : (rt + 1) * QT], ps[:])

        def proj_v(b):
            # natural layout: psum[r, dh2] = sum_d x^T[d, r] W^T[d, dh2]
            for rt in range(NBRT):
                rlo = b * N + rt * QT
                xt = xpool.tile([128, KC, QT], bf16, tag="xv", name="xv")
                nc.sync.dma_start(xt[:], _fold(xv[:, rlo : rlo + QT]))
                for rs in range(QT // 128):
                    ps = ps_proj.tile([128, DHC], f32, tag="proj", name="psv")
                    for kc in range(KC):
                        nc.tensor.matmul(
                            ps[:],
                            xt[:, kc, rs * 128 : (rs + 1) * 128],
                            wv_sb[:, kc, :],
                            start=(kc == 0), stop=False,
                        )
                    nc.tensor.matmul(
                        ps[:], ones_r[:], bv_sb[:], start=False, stop=True
                    )
                    kt = rt * (QT // 128) + rs  # key tile index within batch
                    for h in range(HPC):
                        nc.vector.tensor_copy(
                            vs[b][:, h * NKT + kt, 0:64],
                            ps[:, 64 * h : 64 * h + 64],
                        )

        def attention(b, xT, qts):
            for qt in qts:
                qs = slice(qt * QT, (qt + 1) * QT)
                pvs = [
                    ps_pv.tile([65, QT], f32, tag="pv", name=f"pv{h}")
                    for h in range(HPC)
                ]
                for kt in range(NKT):
                    ks = slice(kt * KT, (kt + 1) * KT)
                    sg = ps_s.tile([128, 2 * QT], f32, tag="sg", name="sg")
                    for h in range(HPC):
                        hp = slice(64 * h, 64 * h + 64)
                        nc.tensor.matmul(
                            sg[:, h * QT : (h + 1) * QT],
                            kTs[b][hp, ks],
                            qTs[b][hp, qs],
                            start=True, stop=True,
                        )
                    pt = ppool.tile([128, 2 * QT], bf16, tag="p", name="pt")
                    nc.scalar.activation(
                        pt[:], sg[:], mybir.ActivationFunctionType.Exp, scale=0.125
                    )
                    for h in range(HPC):
                        nc.tensor.matmul(
                            pvs[h][:],
                            vs[b][:, h * NKT + kt, :],
                            pt[:, h * QT : (h + 1) * QT],
                            start=(kt == 0), stop=(kt == NKT - 1),
                        )
                for h in range(HPC):
                    # copy to SBUF immediately -> frees the psum slot so the
                    # next q-tile's PV starts without a long PE stall
                    pvsb = npool.tile([65, QT], f32, tag="pvsb", name=f"pvsb{h}")
                    nc.vector.tensor_copy(pvsb[:], pvs[h][:])
                    # sumexp row sits at partition 64; shift to 0 via sbuf DMA
                    rc = npool.tile([1, QT], f32, tag="rc", name=f"rc{h}")
                    nc.sync.dma_start(rc[:], pvsb[64:65, :])
                    nc.vector.reciprocal(rc[:], rc[:])
                    rb = npool.tile([64, QT], f32, tag="rb", name=f"rb{h}")
                    nc.gpsimd.partition_broadcast(rb[:], rc[:])
                    if h == 0:
                        nc.vector.tensor_mul(xT[0:64, qs], pvsb[0:64, :], rb[:])
                    else:
                        tmp = npool.tile([64, QT], bf16, tag="tmp", name="tmp")
                        nc.vector.tensor_mul(tmp[:], pvsb[0:64, :], rb[:])
                        nc.sync.dma_start(xT[64:128, qs], tmp[:])
                for ot in range(KC):
                    ps = ps_proj.tile([128, QT], f32, tag="proj", name="pso")
                    nc.tensor.matmul(
                        ps[:],
                        wo_sb[:, ot * 128 : (ot + 1) * 128],
                        xT[:, qs],
                        start=True, stop=True,
                    )
                    ob = opool.tile([128, QT], f32, tag="o", name="ob")
                    nc.vector.tensor_copy(ob[:], ps[:])
                    nc.gpsimd.dma_start(
                        outT[
                            ot * 128 : (ot + 1) * 128,
                            b * N + qt * QT : b * N + (qt + 1) * QT,
                        ],
                        ob[:],
                    )

        xTs = [
            opool.tile([128, N], bf16, tag="xT", name=f"xT{b}", bufs=2)
            for b in range(B)
        ]
        proj_qk(0)
        proj_v(0)
        attention(0, xTs[0], range(0, 2))
        proj_qk(1)
        proj_v(1)
        attention(0, xTs[0], range(2, NQT))
        attention(1, xTs[1], range(0, NQT))

    nc.compile()
    _cache["nc"] = nc
    return nc


def kernel(x_q, x_k, x_v, Wq, bq, Wk, bk, Wv, bv, Wo, bo, _trace=False):
    x_q = np.asarray(x_q, dtype=np.float32)
    x_k = np.asarray(x_k, dtype=np.float32)
    x_v = np.asarray(x_v, dtype=np.float32)
    Wq, Wk, Wv, Wo = (np.asarray(w, dtype=np.float32) for w in (Wq, Wk, Wv, Wo))
    bq, bk, bv, bo = (np.asarray(v, dtype=np.float32) for v in (bq, bk, bv, bo))

    bf = ml_dtypes.bfloat16
    xqT = np.ascontiguousarray(x_q.reshape(R, D).T).astype(bf)
    xkT = np.ascontiguousarray(x_k.reshape(R, D).T).astype(bf)
    xvT = np.ascontiguousarray(x_v.reshape(R, D).T).astype(bf)

    in_maps = []
    for c in range(NC):
        s = slice(DHC * c, DHC * (c + 1))
        in_maps.append(
            {
                "xqT": xqT,
                "xkT": xkT,
                "xvT": xvT,
                "wqT": np.ascontiguousarray(Wq[s, :].T).astype(bf),
                "wkT": np.ascontiguousarray(Wk[s, :].T).astype(bf),
                "wvT": np.ascontiguousarray(Wv[s, :].T).astype(bf),
                "woT": np.ascontiguousarray(Wo[:, s].T).astype(bf),
                "bq": bq[s][None, :].astype(bf),
                "bk": bk[s][None, :].astype(bf),
                "bv": bv[s][None, :].astype(bf),
            }
        )

    nc = build()
    res = run_bass_kernel_spmd(nc, in_maps, core_ids=list(range(NC)), trace=_trace)
    total = np.zeros((D, R), dtype=np.float32)
    for c in range(NC):
        total += res.results[c]["outT"]
    out = total.T + bo[None, :]
    if _trace:
        kernel.last_exec_time_ns = res.exec_time_ns
    return out.reshape(B, N, D).astype(np.float32)
